# revision 11
# baseline (speedup 1.0000x reference)
"""Trainium2 kernel for nn_DictionaryLearning (FISTA loop, 30 iterations).

Math (per data column; columns independent -> data-parallel across 8
cores, 4096 columns each):

    P_m = operator_m @ D ; G_m = P_m^T P_m ; lip = max_m ||G_m||_F
    step = 1/lip ; thr = step*lambd ; A = I - step*G ; b = step*P^T y
    it_{k+1} = shrink(A @ mov_k + b, thr)
    mov_k = (1+mu_k) it_k - mu_k it_{k-1}          (returns mov_30)

Device mapping (v4: hardware For_i loop, static-instruction-minimal):
  On this stack the wall time is ~60us per STATIC instruction in the
  NEFF (instruction upload/dispatch per run), while dynamic re-execution
  inside a hardware For_i loop is ~free. The baseline (unrolled python
  loop, ~1300 static instructions) ran 74-84ms for that reason alone.
  This version wraps the whole FISTA iteration in one For_i hardware
  loop (2-unrolled for buffer ping-pong), reducing the program to ~90
  static instructions:

  * X_all/Y_all [128, 4*4096] f32r ping-pong iterate buffers shared by
    all 4 matrices; b_all [128, 4*4096] f32; w_all [128, 4*128] f32r.
  * per loop iteration k (one body instance):
      - one merged 16384-wide MOMBINE (DVE, in-place: mov overwrites
        the dead it_{k-1} buffer), momentum scalars read from
        [128, 32] tables indexed by the loop variable (ds dynamic AP);
      - per matrix: 8 f32r matmuls (512 cols each) into a full-PSUM
        [128, 4096] tile, then one 4096-wide SHRINK_AFFS
        it_{k+1} = shrink(psum + b, thr) written over the mov buffer;
  * epilogue: one merged MOMBINE for the final extrapolation + 1 DMA.

  f32r matmul precision gives ~1e-2 max-rel error (same math as the
  74ms baseline, gate 2e-2).
"""

import os as _os
import sys

if "/opt/trn_rl_repo" not in sys.path:
    sys.path.insert(0, "/opt/trn_rl_repo")

import numpy as np

import concourse.bacc as bacc
import concourse.mybir as mybir
import concourse.tile as tile
from concourse import bass_utils
from concourse.bass import ds
from concourse.dve_ops import (
    OPS,
    CUSTOM_DVE_SPECS,
    _SUB_OPCODE_FOR_NAME,
    DveOp,
    has_src1,
)
from concourse.dve_spec import Spec, Src0, Src1, C0, C1, C2, maxx, minn, lower
from concourse.dve_uop import DveOpSpec

LAMBD = 0.1
N_CORES = 8
M_MAT, DY, DX = 4, 64, 128
N_DATA = 32768
NSH = N_DATA // N_CORES        # 4096 columns per core
SUB = 512                      # columns per matmul (one PSUM bank, fp32)
F32 = mybir.dt.float32
F32R = mybir.dt.float32r


def _register(name, spec, subdim=False):
    """Register a custom DVE op with self-pinned uop shas."""
    if name in _SUB_OPCODE_FOR_NAME:
        return next(op for op in OPS if op.name == name)
    shas = {}
    for ver in ("v3", "v4"):
        s = DveOpSpec(name=name, opcode=0, uops=lower(spec, ver=ver),
                      rd1_en=has_src1(spec))
        shas[ver] = s.sha(ver)
    op = DveOp(name, spec, subdim=subdim, uops_sha=shas)
    OPS.append(op)
    _SUB_OPCODE_FOR_NAME[name] = max(_SUB_OPCODE_FOR_NAME.values()) + 1
    assert _SUB_OPCODE_FOR_NAME[name] < 0x20
    CUSTOM_DVE_SPECS[name] = spec
    return op


# out = C2 * (zh - clamp(zh, -C1, C1)) with zh = in0 + C0*in1
# (C0=1, C2=1 -> out = softshrink(psum + b, C1))
SHRINK_AFFS = _register(
    "SHRINK_AFFS",
    Spec(
        body=(lambda z: (z - maxx(minn(z, C1), -C1)) * C2)(Src0 + C0 * Src1),
        reference=lambda in0, in1, s0, s1, imm2: (
            lambda z: ((z - np.maximum(np.minimum(z, s1), -s1)) * imm2).astype(
                np.float32
            )
        )(in0 + s0 * in1),
    ),
)

# out = s0*in0 + s1*in1   (momentum combine / final extrapolation)
MOMBINE = _register(
    "MOMBINE",
    Spec(
        body=C0 * Src0 + C1 * Src1,
        reference=lambda in0, in1, s0, s1, imm2: (s0 * in0 + s1 * in1).astype(
            np.float32
        ),
    ),
)


def _host_precompute(y, operator, D, max_iter):
    """Mirror the reference's fp32 scalar/matrix computations in numpy."""
    y = np.asarray(y, np.float32)
    operator = np.asarray(operator, np.float32)
    D = np.asarray(D, np.float32)

    prod = operator @ D                                   # (M, 64, 128)
    gram = np.einsum("mij,mik->mjk", prod, prod).astype(np.float32)
    lip = np.sqrt((gram ** 2).sum(axis=(1, 2))).max()
    step = np.float32(1.0) / np.float32(lip)
    thr = float(np.float32(step * np.float32(LAMBD)))

    A = np.eye(DX, dtype=np.float32)[None] - step * gram  # (M, 128, 128)
    # b = step * P^T y  -> packed (128, M*NSH) per core later
    b = (step * np.matmul(prod.transpose(0, 2, 1), y)).astype(np.float32)

    # lhsT = A^T per matrix (A symmetric), packed (128, M*128)
    wts = np.ascontiguousarray(
        np.transpose(A, (0, 2, 1)).transpose(1, 0, 2).reshape(DX, M_MAT * DX))

    ts = [np.float32(1.0)]
    for _ in range(max_iter + 1):
        ts.append(np.float32(0.5 * (1.0 + np.sqrt(1.0 + 4.0 * ts[-1] ** 2))))
    mus = [0.0] + [
        float(np.float32((ts[k] - 1.0) / ts[k + 1])) for k in range(max_iter)
    ]
    return b, wts, thr, mus


def _build_nc(max_iter, thr, mus, repeat=1, nsh=NSH, sub=SUB):
    """Per-core bass module (SPMD across 8 cores), For_i hardware loop."""
    nc = bacc.Bacc(None, target_bir_lowering=False)
    wide = M_MAT * nsh
    b_d = nc.dram_tensor("b", (DX, wide), F32, kind="ExternalInput")
    w_d = nc.dram_tensor("wts", (DX, M_MAT * DX), F32R, kind="ExternalInput")
    mp_d = nc.dram_tensor("muP", (DX, 32), F32, kind="ExternalInput")
    mn_d = nc.dram_tensor("muN", (DX, 32), F32, kind="ExternalInput")
    o_d = nc.dram_tensor("out", (DX, wide), F32, kind="ExternalOutput")

    n_trip = max_iter // 2
    peel = max_iter % 2
    mu_f = mus[max_iter]

    with tile.TileContext(nc) as tc:
        with (
            tc.tile_pool(name="sb", bufs=1) as sb,
            tc.tile_pool(name="ps", bufs=1, space="PSUM") as ps_pool,
        ):
            bt = sb.tile([DX, wide], F32, tag="b", name="b")
            wt = sb.tile([DX, M_MAT * DX], F32R, tag="w", name="w")
            mpt = sb.tile([DX, 32], F32, tag="mp", name="mp")
            mnt = sb.tile([DX, 32], F32, tag="mn", name="mn")
            X = sb.tile([DX, wide], F32R, tag="X", name="X")
            Y = sb.tile([DX, wide], F32R, tag="Y", name="Y")
            pc = ps_pool.tile([DX, nsh], F32, tag="z", name="z")

            for _ in range(repeat):
                nc.sync.dma_start(bt[:], b_d[:])
                nc.sync.dma_start(wt[:], w_d[:])
                nc.sync.dma_start(mpt[:], mp_d[:])
                nc.sync.dma_start(mnt[:], mn_d[:])
                for half in range(2):
                    hs = slice(half * wide // 2, (half + 1) * wide // 2)
                    nc.vector.memset(X[:, hs].bitcast(F32), 0)
                    nc.vector.memset(Y[:, hs].bitcast(F32), 0)

                def iteration(cur, prev, s0_ap, s1_ap):
                    """One FISTA iteration: prev <- it_next.
                    cur holds it_k, prev holds it_{k-1} (overwritten by
                    mov then by it_{k+1})."""
                    nc.vector._custom_dve(
                        MOMBINE, out=prev[:], in0=cur[:], in1=prev[:],
                        s0=s0_ap, s1=s1_ap)
                    for m in range(M_MAT):
                        wsl = wt[:, m * DX:(m + 1) * DX]
                        for s in range(nsh // sub):
                            col = m * nsh + s * sub
                            nc.tensor.matmul(
                                pc[:, s * sub:(s + 1) * sub],
                                wsl, prev[:, col:col + sub],
                                start=True, stop=True)
                        msl = slice(m * nsh, (m + 1) * nsh)
                        nc.vector._custom_dve(
                            SHRINK_AFFS, out=prev[:, msl], in0=pc[:],
                            in1=bt[:, msl], s0=1.0, s1=thr, imm2=1.0)

                with tc.For_i(0, 2 * n_trip, 2) as j:
                    iteration(X, Y, mpt[:, ds(j, 1)], mnt[:, ds(j, 1)])
                    iteration(Y, X, mpt[:, ds(j + 1, 1)],
                              mnt[:, ds(j + 1, 1)])
                # after even #iters: X holds it_{last}, Y holds it_{last-1}
                cur, prev = X, Y
                if peel:
                    iteration(cur, prev, float(1.0 + mus[max_iter - 1]),
                              float(-mus[max_iter - 1]))
                    cur, prev = prev, cur
                # out = (1+mu_f) it_n - mu_f it_{n-1}  (over prev, dead)
                nc.vector._custom_dve(
                    MOMBINE, out=prev[:], in0=cur[:], in1=prev[:],
                    s0=float(1.0 + mu_f), s1=float(-mu_f))
                nc.sync.dma_start(o_d[:], prev[:].bitcast(F32))
    nc.compile()
    return nc


_NC_CACHE = {}


def _get_nc(max_iter, thr, mus, repeat=1):
    key = (max_iter, float(thr), repeat)
    if key not in _NC_CACHE:
        _NC_CACHE[key] = _build_nc(max_iter, thr, mus, repeat)
    return _NC_CACHE[key]


def kernel(y, operator, D, max_iter, _repeat=1):
    max_iter = int(max_iter)
    y = np.asarray(y, np.float32)
    assert y.shape == (M_MAT, DY, N_DATA) and max_iter >= 2

    b, wts, thr, mus = _host_precompute(y, operator, D, max_iter)
    nc = _get_nc(max_iter, thr, mus, _repeat)

    muP = np.zeros((DX, 32), np.float32)
    muN = np.zeros((DX, 32), np.float32)
    for k in range(max_iter):
        muP[:, k] = np.float32(1.0 + mus[k])
        muN[:, k] = np.float32(-mus[k])

    in_maps = []
    for c in range(N_CORES):
        sl = slice(c * NSH, (c + 1) * NSH)
        # pack (M, 128, nsh) -> (128, M*nsh)
        bc = np.ascontiguousarray(
            b[:, :, sl].transpose(1, 0, 2).reshape(DX, M_MAT * NSH))
        in_maps.append({
            "b": bc,
            "wts": wts,
            "muP": muP,
            "muN": muN,
        })
    res = bass_utils.run_bass_kernel_spmd(nc, in_maps,
                                          core_ids=list(range(N_CORES)))
    out = np.stack([
        res.results[c]["out"].reshape(DX, M_MAT, NSH).transpose(1, 0, 2)
        for c in range(N_CORES)
    ], axis=0)  # (cores, M, DX, NSH)
    out = np.concatenate([out[c] for c in range(N_CORES)], axis=2)
    return out.astype(np.float32)
